# revision 1
# baseline (speedup 1.0000x reference)
"""Trainium2 Bass kernel for GQA attention prefill (Mistral-style, RoPE, causal).

B=1, S=2048, DIM=4096, 32 Q heads / 8 KV heads, HD=128, rope theta 1e6.

Sharding: tensor-parallel over heads across 8 cores. Core i gets Q heads
4i..4i+3 and KV head i. x is replicated (pre-transposed + bf16-cast on host).
Each core computes its 4 heads' attention and a partial output projection
(contraction over its 512 input dims of wo); the host sums the 8 partials.

Per-core dataflow (all matmuls bf16 with fp32 PSUM accumulation):
  phase A (per 128-row s block):
    xT tiles [c,s] (lhsT) x wT [c, q|k|v] (rhs) -> psum [s, 768]
    rope applied in [s, d] layout via stride-2 APs (DVE), cast bf16
    PE-transpose q/k 128x128 blocks -> resident QT/KT [d, s]; V kept [s, d]
  phase B (per 512-col q chunk t, per head h):
    scores_T [k,q] = KT_tile.T @ QT  (one matmul per 128-k tile, no accum)
    P_T = exp(scale * scores_T) on ACT (no max subtraction: |scores| < ~15),
    diagonal blocks masked by precomputed 0/1 tiles (DVE)
    attn_T [d, q] accumulated via lhsT=V tiles; denom via lhsT=ones
    (every row of the denom psum equals the column sum of P_T)
    normalize on DVE (reciprocal + multiply) -> at [d', s] bf16
  o-proj: psum [s,512e] accumulated over the 4 heads, lhsT=at slices,
    rhs=woT [d', e]; evacuate fp32 and DMA to the partial output.
"""

import numpy as np
import ml_dtypes

S = 2048
DIM = 4096
HD = 128
N_CORES = 8
QH_PER_CORE = 4  # 512 q dims per core
DQ = QH_PER_CORE * HD  # 512
SCALE = 1.0 / float(np.sqrt(HD))
SB = S // 128  # 16 s blocks
CB = DIM // 128  # 32 contraction blocks
NT = S // 512  # 4 q chunks
ET = DIM // 512  # 8 e tiles

bf16 = ml_dtypes.bfloat16

_RUNNER = None


def _build():
    import concourse.bass as bass
    import concourse.mybir as mybir
    import concourse.tile as tile
    from concourse import bacc
    from concourse.masks import make_identity

    dt = mybir.dt
    Exp = mybir.ActivationFunctionType.Exp

    nc = bacc.Bacc(
        "TRN2", target_bir_lowering=False, debug=False, num_devices=N_CORES
    )

    xt_d = nc.dram_tensor("xt", [DIM, S], dt.bfloat16, kind="ExternalInput").ap()
    wt_d = nc.dram_tensor("wt", [DIM, 768], dt.bfloat16, kind="ExternalInput").ap()
    wot_d = nc.dram_tensor("wot", [DQ, DIM], dt.bfloat16, kind="ExternalInput").ap()
    cs4_d = nc.dram_tensor("cs4", [S, 256], dt.float32, kind="ExternalInput").ap()
    sn4_d = nc.dram_tensor("sn4", [S, 256], dt.float32, kind="ExternalInput").ap()
    mask_d = nc.dram_tensor("mask", [512, 512], dt.bfloat16, kind="ExternalInput").ap()
    out_d = nc.dram_tensor("out", [S, DIM], dt.float32, kind="ExternalOutput").ap()

    with tile.TileContext(nc) as tc:
        with tc.tile_pool(name="const", bufs=1) as cp:
            wt_sb = cp.tile([128, CB, 768], dt.bfloat16)
            nc.sync.dma_start(out=wt_sb, in_=wt_d.rearrange("(cb c) n -> c cb n", c=128))
            woT_sb = cp.tile([128, QH_PER_CORE, DIM], dt.bfloat16)
            nc.sync.dma_start(
                out=woT_sb, in_=wot_d.rearrange("(db p) e -> p db e", p=128)
            )
            cs4_sb = cp.tile([128, SB, 256], dt.float32)
            nc.sync.dma_start(out=cs4_sb, in_=cs4_d.rearrange("(sb p) n -> p sb n", p=128))
            sn4_sb = cp.tile([128, SB, 256], dt.float32)
            nc.sync.dma_start(out=sn4_sb, in_=sn4_d.rearrange("(sb p) n -> p sb n", p=128))
            mask_sb = cp.tile([128, 4, 512], dt.bfloat16)
            nc.sync.dma_start(out=mask_sb, in_=mask_d.rearrange("(m p) n -> p m n", p=128))
            ones_sb = cp.tile([128, 128], dt.bfloat16)
            nc.vector.memset(ones_sb, 1.0)
            ident_sb = cp.tile([128, 128], dt.bfloat16)
            make_identity(nc, ident_sb)

            qt_sb = cp.tile([128, QH_PER_CORE, S], dt.bfloat16)  # [d, h, s]
            kt_sb = cp.tile([128, S], dt.bfloat16)  # [d, s]
            v_sb = cp.tile([128, SB, HD], dt.bfloat16)  # [s128, sb, d]

            # ---------------- phase A: projections + rope + transposes --------
            with (
                tc.tile_pool(name="pa", bufs=2) as pa,
                tc.tile_pool(name="pap", bufs=2, space="PSUM") as pap,
            ):
                for sb in range(SB):
                    xt_sb = pa.tile([128, CB, 128], dt.bfloat16, tag="xt")
                    nc.sync.dma_start(
                        out=xt_sb,
                        in_=xt_d.rearrange("(cb c) s -> c cb s", c=128)[
                            :, :, sb * 128 : (sb + 1) * 128
                        ],
                    )
                    ps = pap.tile([128, 768], dt.float32, tag="proj")
                    for cb in range(CB):
                        nc.tensor.matmul(
                            ps[:, 0:512],
                            lhsT=xt_sb[:, cb, :],
                            rhs=wt_sb[:, cb, 0:512],
                            start=(cb == 0),
                            stop=(cb == CB - 1),
                        )
                        nc.tensor.matmul(
                            ps[:, 512:768],
                            lhsT=xt_sb[:, cb, :],
                            rhs=wt_sb[:, cb, 512:768],
                            start=(cb == 0),
                            stop=(cb == CB - 1),
                        )
                    # rope Q (4 heads at once) then K, in [s, d] layout
                    q_sb = pa.tile([128, DQ], dt.bfloat16, tag="q")
                    k_sb = pa.tile([128, HD], dt.bfloat16, tag="k")
                    t1 = pa.tile([128, 256], dt.float32, tag="t1")
                    t2 = pa.tile([128, 256], dt.float32, tag="t2")
                    c4 = cs4_sb[:, sb, :]
                    s4 = sn4_sb[:, sb, :]
                    a = ps[:, 0:512:2]
                    b = ps[:, 1:512:2]
                    nc.vector.tensor_mul(t1, a, c4)
                    nc.vector.tensor_mul(t2, b, s4)
                    nc.vector.tensor_sub(q_sb[:, 0:512:2], t1, t2)
                    nc.vector.tensor_mul(t1, a, s4)
                    nc.vector.tensor_mul(t2, b, c4)
                    nc.vector.tensor_add(q_sb[:, 1:512:2], t1, t2)
                    ak = ps[:, 512:640:2]
                    bk = ps[:, 513:640:2]
                    t1k = t1[:, 0:64]
                    t2k = t2[:, 0:64]
                    c4k = c4[:, 0:64]
                    s4k = s4[:, 0:64]
                    nc.vector.tensor_mul(t1k, ak, c4k)
                    nc.vector.tensor_mul(t2k, bk, s4k)
                    nc.vector.tensor_sub(k_sb[:, 0:128:2], t1k, t2k)
                    nc.vector.tensor_mul(t1k, ak, s4k)
                    nc.vector.tensor_mul(t2k, bk, c4k)
                    nc.vector.tensor_add(k_sb[:, 1:128:2], t1k, t2k)
                    # V: straight cast copy
                    nc.vector.tensor_copy(v_sb[:, sb, :], ps[:, 640:768])
                    # transposes into resident QT / KT
                    for h in range(QH_PER_CORE):
                        pst = pap.tile([128, 128], dt.bfloat16, tag="tp", bufs=4)
                        nc.tensor.transpose(
                            pst, q_sb[:, h * 128 : (h + 1) * 128], ident_sb
                        )
                        nc.vector.tensor_copy(
                            qt_sb[:, h, sb * 128 : (sb + 1) * 128], pst
                        )
                    pst = pap.tile([128, 128], dt.bfloat16, tag="tp", bufs=4)
                    nc.tensor.transpose(pst, k_sb, ident_sb)
                    nc.vector.tensor_copy(kt_sb[:, sb * 128 : (sb + 1) * 128], pst)

            # ---------------- phase B: attention + output projection ----------
            with (
                tc.tile_pool(name="pb", bufs=2) as pb,
                tc.tile_pool(name="pbp", bufs=2, space="PSUM") as pbp,
            ):
                for t in range(NT):
                    nkb = 4 * (t + 1)
                    at_tiles = []
                    for h in range(QH_PER_CORE):
                        qs = qt_sb[:, h, t * 512 : (t + 1) * 512]
                        ps_o = pbp.tile([128, 512], dt.float32, tag="attnT")
                        ps_d = pbp.tile([128, 512], dt.float32, tag="denom")
                        for kb in range(nkb):
                            ps_s = pbp.tile([128, 512], dt.float32, tag="scores")
                            nc.tensor.matmul(
                                ps_s,
                                lhsT=kt_sb[:, kb * 128 : (kb + 1) * 128],
                                rhs=qs,
                                start=True,
                                stop=True,
                            )
                            pt = pb.tile([128, 512], dt.bfloat16, tag="pt", bufs=4)
                            nc.scalar.activation(pt, ps_s, Exp, scale=SCALE)
                            if kb >= 4 * t:
                                nc.vector.tensor_mul(
                                    pt, pt, mask_sb[:, kb - 4 * t, :]
                                )
                            nc.tensor.matmul(
                                ps_o,
                                lhsT=v_sb[:, kb, :],
                                rhs=pt,
                                start=(kb == 0),
                                stop=(kb == nkb - 1),
                            )
                            nc.tensor.matmul(
                                ps_d,
                                lhsT=ones_sb,
                                rhs=pt,
                                start=(kb == 0),
                                stop=(kb == nkb - 1),
                            )
                        recip = pb.tile([128, 512], dt.float32, tag="recip")
                        nc.vector.reciprocal(recip, ps_d)
                        at = pb.tile([128, 512], dt.bfloat16, tag=f"at{h}")
                        nc.vector.tensor_mul(at, ps_o, recip)
                        at_tiles.append(at)
                    for sb in range(4):
                        for e in range(ET):
                            ps_out = pbp.tile([128, 512], dt.float32, tag="oproj")
                            for h in range(QH_PER_CORE):
                                nc.tensor.matmul(
                                    ps_out,
                                    lhsT=at_tiles[h][:, sb * 128 : (sb + 1) * 128],
                                    rhs=woT_sb[:, h, e * 512 : (e + 1) * 512],
                                    start=(h == 0),
                                    stop=(h == QH_PER_CORE - 1),
                                )
                            o_sb = pb.tile([128, 512], dt.float32, tag="osb", bufs=6)
                            nc.vector.tensor_copy(o_sb, ps_out)
                            nc.sync.dma_start(
                                out=out_d[
                                    (4 * t + sb) * 128 : (4 * t + sb + 1) * 128,
                                    e * 512 : (e + 1) * 512,
                                ],
                                in_=o_sb,
                            )

    nc.compile()
    return nc


def _prep_inputs(x, cos, sin, wq, wk, wv, wo):
    x = np.asarray(x, dtype=np.float32)
    cos = np.asarray(cos, dtype=np.float32)
    sin = np.asarray(sin, dtype=np.float32)
    wq = np.asarray(wq, dtype=np.float32)
    wk = np.asarray(wk, dtype=np.float32)
    wv = np.asarray(wv, dtype=np.float32)
    wo = np.asarray(wo, dtype=np.float32)

    xt = np.ascontiguousarray(x[0].T).astype(bf16)  # [DIM, S]
    cs4 = np.ascontiguousarray(np.tile(cos, (1, 4)))  # [S, 256] f32
    sn4 = np.ascontiguousarray(np.tile(sin, (1, 4)))

    # causal masks for the 4 diagonal sub-blocks: mask[r, c] = (r + delta) <= c
    r = np.arange(128)[:, None]
    c = np.arange(512)[None, :]
    mask = np.concatenate(
        [((r + d) <= c).astype(bf16) for d in (0, 128, 256, 384)], axis=0
    )  # [512, 512]

    in_maps = []
    for i in range(N_CORES):
        wq_i = wq[DQ * i : DQ * (i + 1)]  # [512, DIM]
        wk_i = wk[HD * i : HD * (i + 1)]  # [128, DIM]
        wv_i = wv[HD * i : HD * (i + 1)]
        wt = np.concatenate([wq_i.T, wk_i.T, wv_i.T], axis=1).astype(bf16)  # [DIM,768]
        wot = np.ascontiguousarray(wo[:, DQ * i : DQ * (i + 1)].T).astype(
            bf16
        )  # [512, DIM]
        in_maps.append(
            {
                "xt": xt,
                "wt": np.ascontiguousarray(wt),
                "wot": wot,
                "cs4": cs4,
                "sn4": sn4,
                "mask": np.ascontiguousarray(mask),
            }
        )
    return in_maps


def _get_runner():
    global _RUNNER
    if _RUNNER is None:
        _RUNNER = _build()
    return _RUNNER


def kernel(x, cos, sin, wq, wk, wv, wo):
    from concourse.bass_utils import run_bass_kernel_spmd

    nc = _get_runner()
    in_maps = _prep_inputs(x, cos, sin, wq, wk, wv, wo)
    res = run_bass_kernel_spmd(nc, in_maps, list(range(N_CORES)))
    out = np.zeros((S, DIM), dtype=np.float32)
    for i in range(N_CORES):
        out += res.results[i]["out"]
    return out[None].astype(np.float32)


# revision 2
# speedup vs baseline: 124.8919x; 124.8919x over previous
"""Trainium2 Bass kernel for GQA attention prefill (Mistral-style, RoPE, causal).

B=1, S=2048, DIM=4096, 32 Q heads / 8 KV heads, HD=128, rope theta 1e6.

Sharding: tensor-parallel over heads across 8 cores. Core i gets Q heads
4i..4i+3 and KV head i. x is replicated (pre-transposed + bf16-cast on host).
Each core computes its 4 heads' attention and a partial output projection
(contraction over its 512 input dims of wo); the host sums the 8 partials.

Per-core dataflow (all matmuls bf16 with fp32 PSUM accumulation):
  phase A (per 128-row s block):
    xT tiles [c,s] (lhsT) x wT [c, q|k|v] (rhs) -> psum [s, 768]
    rope applied in [s, d] layout via stride-2 APs (DVE), cast bf16
    PE-transpose q/k 128x128 blocks -> resident QT/KT [d, s]; V kept [s, d]
  phase B (per 512-col q chunk t, per head h):
    scores_T [k,q] = KT_tile.T @ QT  (one matmul per 128-k tile, no accum)
    P_T = exp(scale * scores_T) on ACT (no max subtraction: |scores| < ~15),
    diagonal blocks masked by precomputed 0/1 tiles (DVE)
    attn_T [d, q] accumulated via lhsT=V tiles; denom via lhsT=ones
    (every row of the denom psum equals the column sum of P_T)
    normalize on DVE (reciprocal + multiply) -> at [d', s] bf16
  o-proj: psum [s,512e] accumulated over the 4 heads, lhsT=at slices,
    rhs=woT [d', e]; evacuate fp32 and DMA to the partial output.
"""

import numpy as np
import ml_dtypes

S = 2048
DIM = 4096
HD = 128
N_CORES = 8
QH_PER_CORE = 4  # 512 q dims per core
DQ = QH_PER_CORE * HD  # 512
SCALE = 1.0 / float(np.sqrt(HD))
SB = S // 128  # 16 s blocks
CB = DIM // 128  # 32 contraction blocks
NT = S // 512  # 4 q chunks
ET = DIM // 512  # 8 e tiles

bf16 = ml_dtypes.bfloat16

_RUNNER = None


def _build(reps=None):
    import concourse.bass as bass
    import concourse.mybir as mybir
    import concourse.tile as tile
    from concourse import bacc
    from concourse.masks import make_identity
    from contextlib import nullcontext

    dt = mybir.dt
    Exp = mybir.ActivationFunctionType.Exp

    nc = bacc.Bacc(
        "TRN2", target_bir_lowering=False, debug=False, num_devices=N_CORES
    )

    xt_d = nc.dram_tensor("xt", [DIM, S], dt.bfloat16, kind="ExternalInput").ap()
    wt_d = nc.dram_tensor("wt", [DIM, 768], dt.bfloat16, kind="ExternalInput").ap()
    wot_d = nc.dram_tensor("wot", [DQ, DIM], dt.bfloat16, kind="ExternalInput").ap()
    cs4_d = nc.dram_tensor("cs4", [S, 256], dt.float32, kind="ExternalInput").ap()
    sn4_d = nc.dram_tensor("sn4", [S, 256], dt.float32, kind="ExternalInput").ap()
    mask_d = nc.dram_tensor("mask", [512, 512], dt.bfloat16, kind="ExternalInput").ap()
    out_d = nc.dram_tensor("out", [S, DIM], dt.float32, kind="ExternalOutput").ap()

    with tile.TileContext(nc) as tc:
        with tc.For_i(0, reps, 1) if reps else nullcontext(), tc.tile_pool(
            name="const", bufs=1
        ) as cp:
            wt_sb = cp.tile([128, CB, 768], dt.bfloat16)
            nc.sync.dma_start(out=wt_sb, in_=wt_d.rearrange("(cb c) n -> c cb n", c=128))
            woT_sb = cp.tile([128, QH_PER_CORE, DIM], dt.bfloat16)
            nc.sync.dma_start(
                out=woT_sb, in_=wot_d.rearrange("(db p) e -> p db e", p=128)
            )
            cs4_sb = cp.tile([128, SB, 256], dt.float32)
            nc.sync.dma_start(out=cs4_sb, in_=cs4_d.rearrange("(sb p) n -> p sb n", p=128))
            sn4_sb = cp.tile([128, SB, 256], dt.float32)
            nc.sync.dma_start(out=sn4_sb, in_=sn4_d.rearrange("(sb p) n -> p sb n", p=128))
            mask_sb = cp.tile([128, 4, 512], dt.bfloat16)
            nc.sync.dma_start(out=mask_sb, in_=mask_d.rearrange("(m p) n -> p m n", p=128))
            ones_sb = cp.tile([128, 128], dt.bfloat16)
            nc.vector.memset(ones_sb, 1.0)
            ident_sb = cp.tile([128, 128], dt.bfloat16)
            make_identity(nc, ident_sb)

            qt_sb = cp.tile([128, QH_PER_CORE, S], dt.bfloat16)  # [d, h, s]
            kt_sb = cp.tile([128, S], dt.bfloat16)  # [d, s]
            v_sb = cp.tile([128, SB, HD], dt.bfloat16)  # [s128, sb, d]

            # ---------------- phase A: projections + rope + transposes --------
            with (
                tc.tile_pool(name="pa", bufs=2) as pa,
                tc.tile_pool(name="pap", bufs=2, space="PSUM") as pap,
            ):
                for sb in range(SB):
                    xt_sb = pa.tile([128, CB, 128], dt.bfloat16, tag="xt")
                    nc.sync.dma_start(
                        out=xt_sb,
                        in_=xt_d.rearrange("(cb c) s -> c cb s", c=128)[
                            :, :, sb * 128 : (sb + 1) * 128
                        ],
                    )
                    ps = pap.tile([128, 768], dt.float32, tag="proj")
                    for cb in range(CB):
                        nc.tensor.matmul(
                            ps[:, 0:512],
                            lhsT=xt_sb[:, cb, :],
                            rhs=wt_sb[:, cb, 0:512],
                            start=(cb == 0),
                            stop=(cb == CB - 1),
                        )
                        nc.tensor.matmul(
                            ps[:, 512:768],
                            lhsT=xt_sb[:, cb, :],
                            rhs=wt_sb[:, cb, 512:768],
                            start=(cb == 0),
                            stop=(cb == CB - 1),
                        )
                    # rope Q (4 heads at once) then K, in [s, d] layout
                    q_sb = pa.tile([128, DQ], dt.bfloat16, tag="q")
                    k_sb = pa.tile([128, HD], dt.bfloat16, tag="k")
                    t1 = pa.tile([128, 256], dt.float32, tag="t1")
                    t2 = pa.tile([128, 256], dt.float32, tag="t2")
                    c4 = cs4_sb[:, sb, :]
                    s4 = sn4_sb[:, sb, :]
                    a = ps[:, 0:512:2]
                    b = ps[:, 1:512:2]
                    nc.vector.tensor_mul(t1, a, c4)
                    nc.vector.tensor_mul(t2, b, s4)
                    nc.vector.tensor_sub(q_sb[:, 0:512:2], t1, t2)
                    nc.vector.tensor_mul(t1, a, s4)
                    nc.vector.tensor_mul(t2, b, c4)
                    nc.vector.tensor_add(q_sb[:, 1:512:2], t1, t2)
                    ak = ps[:, 512:640:2]
                    bk = ps[:, 513:640:2]
                    t1k = t1[:, 0:64]
                    t2k = t2[:, 0:64]
                    c4k = c4[:, 0:64]
                    s4k = s4[:, 0:64]
                    nc.vector.tensor_mul(t1k, ak, c4k)
                    nc.vector.tensor_mul(t2k, bk, s4k)
                    nc.vector.tensor_sub(k_sb[:, 0:128:2], t1k, t2k)
                    nc.vector.tensor_mul(t1k, ak, s4k)
                    nc.vector.tensor_mul(t2k, bk, c4k)
                    nc.vector.tensor_add(k_sb[:, 1:128:2], t1k, t2k)
                    # V: straight cast copy
                    nc.vector.tensor_copy(v_sb[:, sb, :], ps[:, 640:768])
                    # transposes into resident QT / KT
                    for h in range(QH_PER_CORE):
                        pst = pap.tile([128, 128], dt.bfloat16, tag="tp", bufs=4)
                        nc.tensor.transpose(
                            pst, q_sb[:, h * 128 : (h + 1) * 128], ident_sb
                        )
                        nc.vector.tensor_copy(
                            qt_sb[:, h, sb * 128 : (sb + 1) * 128], pst
                        )
                    pst = pap.tile([128, 128], dt.bfloat16, tag="tp", bufs=4)
                    nc.tensor.transpose(pst, k_sb, ident_sb)
                    nc.vector.tensor_copy(kt_sb[:, sb * 128 : (sb + 1) * 128], pst)

            # ---------------- phase B: attention + output projection ----------
            with (
                tc.tile_pool(name="pb", bufs=2) as pb,
                tc.tile_pool(name="pbp", bufs=2, space="PSUM") as pbp,
            ):
                for t in range(NT):
                    nkb = 4 * (t + 1)
                    at_tiles = []
                    for h in range(QH_PER_CORE):
                        qs = qt_sb[:, h, t * 512 : (t + 1) * 512]
                        ps_o = pbp.tile([128, 512], dt.float32, tag="attnT")
                        ps_d = pbp.tile([128, 512], dt.float32, tag="denom")
                        for kb in range(nkb):
                            ps_s = pbp.tile([128, 512], dt.float32, tag="scores")
                            nc.tensor.matmul(
                                ps_s,
                                lhsT=kt_sb[:, kb * 128 : (kb + 1) * 128],
                                rhs=qs,
                                start=True,
                                stop=True,
                            )
                            pt = pb.tile([128, 512], dt.bfloat16, tag="pt", bufs=4)
                            nc.scalar.activation(pt, ps_s, Exp, scale=SCALE)
                            if kb >= 4 * t:
                                nc.vector.tensor_mul(
                                    pt, pt, mask_sb[:, kb - 4 * t, :]
                                )
                            nc.tensor.matmul(
                                ps_o,
                                lhsT=v_sb[:, kb, :],
                                rhs=pt,
                                start=(kb == 0),
                                stop=(kb == nkb - 1),
                            )
                            nc.tensor.matmul(
                                ps_d,
                                lhsT=ones_sb,
                                rhs=pt,
                                start=(kb == 0),
                                stop=(kb == nkb - 1),
                            )
                        recip = pb.tile([128, 512], dt.float32, tag="recip")
                        nc.vector.reciprocal(recip, ps_d)
                        at = pb.tile([128, 512], dt.bfloat16, tag=f"at{h}")
                        nc.vector.tensor_mul(at, ps_o, recip)
                        at_tiles.append(at)
                    for sb in range(4):
                        for e in range(ET):
                            ps_out = pbp.tile([128, 512], dt.float32, tag="oproj")
                            for h in range(QH_PER_CORE):
                                nc.tensor.matmul(
                                    ps_out,
                                    lhsT=at_tiles[h][:, sb * 128 : (sb + 1) * 128],
                                    rhs=woT_sb[:, h, e * 512 : (e + 1) * 512],
                                    start=(h == 0),
                                    stop=(h == QH_PER_CORE - 1),
                                )
                            o_sb = pb.tile([128, 512], dt.float32, tag="osb", bufs=6)
                            nc.vector.tensor_copy(o_sb, ps_out)
                            nc.sync.dma_start(
                                out=out_d[
                                    (4 * t + sb) * 128 : (4 * t + sb + 1) * 128,
                                    e * 512 : (e + 1) * 512,
                                ],
                                in_=o_sb,
                            )

    nc.compile()
    return nc


def _prep_inputs(x, cos, sin, wq, wk, wv, wo):
    x = np.asarray(x, dtype=np.float32)
    cos = np.asarray(cos, dtype=np.float32)
    sin = np.asarray(sin, dtype=np.float32)
    wq = np.asarray(wq, dtype=np.float32)
    wk = np.asarray(wk, dtype=np.float32)
    wv = np.asarray(wv, dtype=np.float32)
    wo = np.asarray(wo, dtype=np.float32)

    xt = np.ascontiguousarray(x[0].T).astype(bf16)  # [DIM, S]
    cs4 = np.ascontiguousarray(np.tile(cos, (1, 4)))  # [S, 256] f32
    sn4 = np.ascontiguousarray(np.tile(sin, (1, 4)))

    # causal masks for the 4 diagonal sub-blocks: mask[r, c] = (r + delta) <= c
    r = np.arange(128)[:, None]
    c = np.arange(512)[None, :]
    mask = np.concatenate(
        [((r + d) <= c).astype(bf16) for d in (0, 128, 256, 384)], axis=0
    )  # [512, 512]

    in_maps = []
    for i in range(N_CORES):
        wq_i = wq[DQ * i : DQ * (i + 1)]  # [512, DIM]
        wk_i = wk[HD * i : HD * (i + 1)]  # [128, DIM]
        wv_i = wv[HD * i : HD * (i + 1)]
        wt = np.concatenate([wq_i.T, wk_i.T, wv_i.T], axis=1).astype(bf16)  # [DIM,768]
        wot = np.ascontiguousarray(wo[:, DQ * i : DQ * (i + 1)].T).astype(
            bf16
        )  # [512, DIM]
        in_maps.append(
            {
                "xt": xt,
                "wt": np.ascontiguousarray(wt),
                "wot": wot,
                "cs4": cs4,
                "sn4": sn4,
                "mask": np.ascontiguousarray(mask),
            }
        )
    return in_maps


def _get_runner():
    global _RUNNER
    if _RUNNER is None:
        _RUNNER = _build()
    return _RUNNER


def kernel(x, cos, sin, wq, wk, wv, wo):
    from concourse.bass_utils import run_bass_kernel_spmd

    nc = _get_runner()
    in_maps = _prep_inputs(x, cos, sin, wq, wk, wv, wo)
    res = run_bass_kernel_spmd(nc, in_maps, list(range(N_CORES)))
    out = np.zeros((S, DIM), dtype=np.float32)
    for i in range(N_CORES):
        out += res.results[i]["out"]
    return out[None].astype(np.float32)


# revision 5
# speedup vs baseline: 129.7351x; 1.0388x over previous
"""Trainium2 Bass kernel for GQA attention prefill (Mistral-style, RoPE, causal).

B=1, S=2048, DIM=4096, 32 Q heads / 8 KV heads, HD=128, rope theta 1e6.

Sharding: tensor-parallel over heads across 8 cores. Core i gets Q heads
4i..4i+3 and KV head i. x is replicated (pre-transposed + bf16-cast on host).
Each core computes its 4 heads' attention and a partial output projection
(contraction over its 512 input dims of wo); the host sums the 8 partials.

Per-core dataflow (all matmuls bf16 with fp32 PSUM accumulation):
  phase A (per 128-row s block):
    xT tiles [c,s] (lhsT) x wT [c, q|k|v] (rhs) -> psum [s, 768]
    rope applied in [s, d] layout via stride-2 APs (DVE), cast bf16
    PE-transpose q/k 128x128 blocks -> resident QT/KT [d, s]; V kept [s, d]
  phase B (per 512-col q chunk t, per head h):
    scores_T [k,q] = KT_tile.T @ QT  (one matmul per 128-k tile, no accum)
    P_T = exp(scale * scores_T) on ACT (no max subtraction: |scores| < ~15),
    diagonal blocks masked by precomputed 0/1 tiles (DVE)
    attn_T [d, q] accumulated via lhsT=V tiles; denom via lhsT=ones
    (every row of the denom psum equals the column sum of P_T)
    normalize on DVE (reciprocal + multiply) -> at [d', s] bf16
  o-proj: psum [s,512e] accumulated over the 4 heads, lhsT=at slices,
    rhs=woT [d', e]; evacuate fp32 and DMA to the partial output.
"""

import numpy as np
import ml_dtypes

S = 2048
DIM = 4096
HD = 128
N_CORES = 8
QH_PER_CORE = 4  # 512 q dims per core
DQ = QH_PER_CORE * HD  # 512
SCALE = 1.0 / float(np.sqrt(HD))
SB = S // 128  # 16 s blocks
CB = DIM // 128  # 32 contraction blocks
NT = S // 512  # 4 q chunks
ET = DIM // 512  # 8 e tiles

bf16 = ml_dtypes.bfloat16

_RUNNER = None


ALL_STAGES = frozenset({"proj", "rope", "tpose", "scores", "pv", "oproj"})


def _build(reps=None, stages=ALL_STAGES):
    import concourse.bass as bass
    import concourse.mybir as mybir
    import concourse.tile as tile
    from concourse import bacc
    from concourse.masks import make_identity
    from contextlib import nullcontext

    dt = mybir.dt
    Exp = mybir.ActivationFunctionType.Exp

    nc = bacc.Bacc(
        "TRN2", target_bir_lowering=False, debug=False, num_devices=N_CORES
    )

    xt_d = nc.dram_tensor("xt", [DIM, S], dt.bfloat16, kind="ExternalInput").ap()
    wt_d = nc.dram_tensor("wt", [DIM, 768], dt.bfloat16, kind="ExternalInput").ap()
    wot_d = nc.dram_tensor("wot", [DQ, DIM], dt.bfloat16, kind="ExternalInput").ap()
    cs4_d = nc.dram_tensor("cs4", [S, 256], dt.float32, kind="ExternalInput").ap()
    sn4_d = nc.dram_tensor("sn4", [S, 256], dt.float32, kind="ExternalInput").ap()
    mask_d = nc.dram_tensor("mask", [512, 512], dt.bfloat16, kind="ExternalInput").ap()
    out_d = nc.dram_tensor("out", [S, DIM], dt.float32, kind="ExternalOutput").ap()

    with tile.TileContext(nc) as tc:
        with tc.For_i(0, reps, 1) if reps else nullcontext(), tc.tile_pool(
            name="const", bufs=1
        ) as cp:
            wt_sb = cp.tile([128, CB, 768], dt.bfloat16)
            nc.sync.dma_start(out=wt_sb, in_=wt_d.rearrange("(cb c) n -> c cb n", c=128))
            cs4_sb = cp.tile([128, SB, 256], dt.float32)
            nc.sync.dma_start(out=cs4_sb, in_=cs4_d.rearrange("(sb p) n -> p sb n", p=128))
            sn4_sb = cp.tile([128, SB, 256], dt.float32)
            nc.sync.dma_start(out=sn4_sb, in_=sn4_d.rearrange("(sb p) n -> p sb n", p=128))
            mask_sb = cp.tile([128, 4, 512], dt.bfloat16)
            nc.sync.dma_start(out=mask_sb, in_=mask_d.rearrange("(m p) n -> p m n", p=128))
            ones_sb = cp.tile([128, 128], dt.bfloat16)
            nc.vector.memset(ones_sb, 1.0)
            ident_sb = cp.tile([128, 128], dt.bfloat16)
            make_identity(nc, ident_sb)

            qt_sb = cp.tile([128, QH_PER_CORE, S], dt.bfloat16)  # [d, h, s]
            kt_sb = cp.tile([128, S], dt.bfloat16)  # [d, s]
            v_sb = cp.tile([128, SB, HD], dt.bfloat16)  # [s128, sb, d]

            # ---------------- phase A: projections + rope + transposes --------
            with (
                tc.tile_pool(name="pa", bufs=2) as pa,
                tc.tile_pool(name="pap", bufs=2, space="PSUM") as pap,
            ):
                for sb in range(SB):
                    if sb % 4 == 0:
                        xt_sb = pa.tile([128, CB, 512], dt.bfloat16, tag="xt")
                        nc.sync.dma_start(
                            out=xt_sb,
                            in_=xt_d.rearrange("(cb c) s -> c cb s", c=128)[
                                :, :, (sb // 4) * 512 : (sb // 4 + 1) * 512
                            ],
                        )
                    sbl = sb % 4
                    ps = pap.tile([128, 768], dt.float32, tag="proj")
                    for cb in range(CB if "proj" in stages else 0):
                        lhsT = xt_sb[:, cb, sbl * 128 : (sbl + 1) * 128]
                        nc.tensor.matmul(
                            ps[:, 0:512],
                            lhsT=lhsT,
                            rhs=wt_sb[:, cb, 0:512],
                            start=(cb == 0),
                            stop=(cb == CB - 1),
                        )
                        nc.tensor.matmul(
                            ps[:, 512:768],
                            lhsT=lhsT,
                            rhs=wt_sb[:, cb, 512:768],
                            start=(cb == 0),
                            stop=(cb == CB - 1),
                        )
                    # rope Q (4 heads at once) then K, in [s, d] layout
                    do_rope = "rope" in stages
                    q_sb = pa.tile([128, DQ], dt.bfloat16, tag="q")
                    k_sb = pa.tile([128, HD], dt.bfloat16, tag="k")
                    t1 = pa.tile([128, 256], dt.float32, tag="t1")
                    t2 = pa.tile([128, 256], dt.float32, tag="t2")
                    c4 = cs4_sb[:, sb, :]
                    s4 = sn4_sb[:, sb, :]
                    a = ps[:, 0:512:2]
                    b = ps[:, 1:512:2]
                    do_rope and nc.vector.tensor_mul(t1, a, c4)
                    do_rope and nc.vector.tensor_mul(t2, b, s4)
                    do_rope and nc.vector.tensor_sub(q_sb[:, 0:512:2], t1, t2)
                    do_rope and nc.vector.tensor_mul(t1, a, s4)
                    do_rope and nc.vector.tensor_mul(t2, b, c4)
                    do_rope and nc.vector.tensor_add(q_sb[:, 1:512:2], t1, t2)
                    ak = ps[:, 512:640:2]
                    bk = ps[:, 513:640:2]
                    t1k = t1[:, 0:64]
                    t2k = t2[:, 0:64]
                    c4k = c4[:, 0:64]
                    s4k = s4[:, 0:64]
                    do_rope and nc.vector.tensor_mul(t1k, ak, c4k)
                    do_rope and nc.vector.tensor_mul(t2k, bk, s4k)
                    do_rope and nc.vector.tensor_sub(k_sb[:, 0:128:2], t1k, t2k)
                    do_rope and nc.vector.tensor_mul(t1k, ak, s4k)
                    do_rope and nc.vector.tensor_mul(t2k, bk, c4k)
                    do_rope and nc.vector.tensor_add(k_sb[:, 1:128:2], t1k, t2k)
                    # V: straight cast copy
                    do_rope and nc.vector.tensor_copy(v_sb[:, sb, :], ps[:, 640:768])
                    # transposes into resident QT / KT
                    for h in range(QH_PER_CORE if "tpose" in stages else 0):
                        pst = pap.tile([128, 128], dt.bfloat16, tag="tp", bufs=4)
                        nc.tensor.transpose(
                            pst, q_sb[:, h * 128 : (h + 1) * 128], ident_sb
                        )
                        nc.vector.tensor_copy(
                            qt_sb[:, h, sb * 128 : (sb + 1) * 128], pst
                        )
                    if "tpose" in stages:
                        pst = pap.tile([128, 128], dt.bfloat16, tag="tp", bufs=4)
                        nc.tensor.transpose(pst, k_sb, ident_sb)
                        nc.vector.tensor_copy(kt_sb[:, sb * 128 : (sb + 1) * 128], pst)

            # ---------------- phase B: attention + output projection ----------
            # Software-pipelined: the o-projection for chunk t-1 is emitted
            # between the per-head attention groups of chunk t, so PE has
            # dense matmul work while ACT runs the exps of the current chunk.
            with (
                tc.tile_pool(name="pb", bufs=2) as pb,
                tc.tile_pool(name="pbp", bufs=2, space="PSUM") as pbp,
            ):
                woT_sb = pb.tile([128, QH_PER_CORE, DIM], dt.bfloat16, bufs=1)
                nc.sync.dma_start(
                    out=woT_sb, in_=wot_d.rearrange("(db p) e -> p db e", p=128)
                )

                def oproj_group(t, sbl, ats):
                    if "oproj" not in stages:
                        return
                    o_sb = pb.tile([128, DIM], dt.float32, tag="osb")
                    for e in range(ET):
                        ps_out = pbp.tile([128, 512], dt.float32, tag="oproj")
                        for h in range(QH_PER_CORE):
                            nc.tensor.matmul(
                                ps_out,
                                lhsT=ats[h][:, sbl * 128 : (sbl + 1) * 128],
                                rhs=woT_sb[:, h, e * 512 : (e + 1) * 512],
                                start=(h == 0),
                                stop=(h == QH_PER_CORE - 1),
                            )
                        nc.vector.tensor_copy(o_sb[:, e * 512 : (e + 1) * 512], ps_out)
                    nc.scalar.dma_start(
                        out=out_d[(4 * t + sbl) * 128 : (4 * t + sbl + 1) * 128, :],
                        in_=o_sb,
                    )

                prev_ats = None
                for t in range(NT if ("scores" in stages) else 0):
                    nkb = 4 * (t + 1)
                    at_tiles = []
                    for h in range(QH_PER_CORE):
                        qs = qt_sb[:, h, t * 512 : (t + 1) * 512]
                        ps_o = pbp.tile([128, 512], dt.float32, tag="attnT")
                        ps_d = pbp.tile([128, 512], dt.float32, tag="denom")
                        for kb in range(nkb):
                            ps_s = pbp.tile([128, 512], dt.float32, tag="scores")
                            nc.tensor.matmul(
                                ps_s,
                                lhsT=kt_sb[:, kb * 128 : (kb + 1) * 128],
                                rhs=qs,
                                start=True,
                                stop=True,
                            )
                            pt = pb.tile([128, 512], dt.bfloat16, tag="pt", bufs=4)
                            nc.scalar.activation(pt, ps_s, Exp, scale=SCALE)
                            if kb >= 4 * t:
                                nc.vector.tensor_mul(
                                    pt, pt, mask_sb[:, kb - 4 * t, :]
                                )
                            if "pv" in stages:
                                nc.tensor.matmul(
                                    ps_o,
                                    lhsT=v_sb[:, kb, :],
                                    rhs=pt,
                                    start=(kb == 0),
                                    stop=(kb == nkb - 1),
                                )
                                nc.tensor.matmul(
                                    ps_d,
                                    lhsT=ones_sb,
                                    rhs=pt,
                                    start=(kb == 0),
                                    stop=(kb == nkb - 1),
                                )
                        at = pb.tile([128, 512], dt.bfloat16, tag=f"at{h}")
                        if "pv" in stages:
                            recip = pb.tile([128, 512], dt.float32, tag="recip")
                            nc.vector.reciprocal(recip, ps_d)
                            nc.vector.tensor_mul(at, ps_o, recip)
                        at_tiles.append(at)
                        if prev_ats is not None:
                            oproj_group(t - 1, h, prev_ats)
                    prev_ats = at_tiles
                if prev_ats is not None:
                    for sbl in range(4):
                        oproj_group(NT - 1, sbl, prev_ats)
    nc.compile()
    return nc


def _prep_inputs(x, cos, sin, wq, wk, wv, wo):
    x = np.asarray(x, dtype=np.float32)
    cos = np.asarray(cos, dtype=np.float32)
    sin = np.asarray(sin, dtype=np.float32)
    wq = np.asarray(wq, dtype=np.float32)
    wk = np.asarray(wk, dtype=np.float32)
    wv = np.asarray(wv, dtype=np.float32)
    wo = np.asarray(wo, dtype=np.float32)

    xt = np.ascontiguousarray(x[0].T).astype(bf16)  # [DIM, S]
    cs4 = np.ascontiguousarray(np.tile(cos, (1, 4)))  # [S, 256] f32
    sn4 = np.ascontiguousarray(np.tile(sin, (1, 4)))

    # causal masks for the 4 diagonal sub-blocks: mask[r, c] = (r + delta) <= c
    r = np.arange(128)[:, None]
    c = np.arange(512)[None, :]
    mask = np.concatenate(
        [((r + d) <= c).astype(bf16) for d in (0, 128, 256, 384)], axis=0
    )  # [512, 512]

    in_maps = []
    for i in range(N_CORES):
        wq_i = wq[DQ * i : DQ * (i + 1)]  # [512, DIM]
        wk_i = wk[HD * i : HD * (i + 1)]  # [128, DIM]
        wv_i = wv[HD * i : HD * (i + 1)]
        wt = np.concatenate([wq_i.T, wk_i.T, wv_i.T], axis=1).astype(bf16)  # [DIM,768]
        wot = np.ascontiguousarray(wo[:, DQ * i : DQ * (i + 1)].T).astype(
            bf16
        )  # [512, DIM]
        in_maps.append(
            {
                "xt": xt,
                "wt": np.ascontiguousarray(wt),
                "wot": wot,
                "cs4": cs4,
                "sn4": sn4,
                "mask": np.ascontiguousarray(mask),
            }
        )
    return in_maps


def _get_runner():
    global _RUNNER
    if _RUNNER is None:
        _RUNNER = _build()
    return _RUNNER


def kernel(x, cos, sin, wq, wk, wv, wo):
    from concourse.bass_utils import run_bass_kernel_spmd

    nc = _get_runner()
    in_maps = _prep_inputs(x, cos, sin, wq, wk, wv, wo)
    res = run_bass_kernel_spmd(nc, in_maps, list(range(N_CORES)))
    out = np.zeros((S, DIM), dtype=np.float32)
    for i in range(N_CORES):
        out += res.results[i]["out"]
    return out[None].astype(np.float32)


# revision 9
# speedup vs baseline: 129.9969x; 1.0020x over previous
"""Trainium2 Bass kernel for GQA attention prefill (Mistral-style, RoPE, causal).

B=1, S=2048, DIM=4096, 32 Q heads / 8 KV heads, HD=128, rope theta 1e6.

Sharding: tensor-parallel over heads across 8 cores. Core i gets Q heads
4i..4i+3 and KV head i. x is replicated (pre-transposed + bf16-cast on host).
Each core computes its 4 heads' attention and a partial output projection
(contraction over its 512 input dims of wo); the host sums the 8 partials.

Per-core dataflow (all matmuls bf16 with fp32 PSUM accumulation):
  phase A (per 128-row s block):
    xT tiles [c,s] (lhsT) x wT [c, q|k|v] (rhs) -> psum [s, 768]
    rope applied in [s, d] layout via stride-2 APs (DVE), cast bf16
    PE-transpose q/k 128x128 blocks -> resident QT/KT [d, s]; V kept [s, d]
  phase B (per 512-col q chunk t, per head h):
    scores_T [k,q] = KT_tile.T @ QT  (one matmul per 128-k tile, no accum)
    P_T = exp(scale * scores_T) on ACT (no max subtraction: |scores| < ~15),
    diagonal blocks masked by precomputed 0/1 tiles (DVE)
    attn_T [d, q] accumulated via lhsT=V tiles; denom via lhsT=ones
    (every row of the denom psum equals the column sum of P_T)
    normalize on DVE (reciprocal + multiply) -> at [d', s] bf16
  o-proj: psum [s,512e] accumulated over the 4 heads, lhsT=at slices,
    rhs=woT [d', e]; evacuate fp32 and DMA to the partial output.
"""

import numpy as np
import ml_dtypes

S = 2048
DIM = 4096
HD = 128
N_CORES = 8
QH_PER_CORE = 4  # 512 q dims per core
DQ = QH_PER_CORE * HD  # 512
SCALE = 1.0 / float(np.sqrt(HD))
SB = S // 128  # 16 s blocks
CB = DIM // 128  # 32 contraction blocks
NT = S // 512  # 4 q chunks
ET = DIM // 512  # 8 e tiles

bf16 = ml_dtypes.bfloat16

_RUNNER = None


ALL_STAGES = frozenset({"proj", "rope", "tpose", "scores", "pv", "oproj"})

# debug knobs for perf isolation (set by bench scripts)
NO_EXP = False
NO_OUTDMA = False
PT_CONST = False


def _build(reps=None, stages=ALL_STAGES):
    import concourse.bass as bass
    import concourse.mybir as mybir
    import concourse.tile as tile
    from concourse import bacc
    from concourse.masks import make_identity
    from contextlib import nullcontext

    dt = mybir.dt
    Exp = mybir.ActivationFunctionType.Exp

    nc = bacc.Bacc(
        "TRN2", target_bir_lowering=False, debug=False, num_devices=N_CORES
    )

    xt_d = nc.dram_tensor("xt", [DIM, S], dt.bfloat16, kind="ExternalInput").ap()
    wt_d = nc.dram_tensor("wt", [DIM, 768], dt.bfloat16, kind="ExternalInput").ap()
    wot_d = nc.dram_tensor("wot", [DQ, DIM], dt.bfloat16, kind="ExternalInput").ap()
    cs4_d = nc.dram_tensor("cs4", [S, 256], dt.float32, kind="ExternalInput").ap()
    sn4_d = nc.dram_tensor("sn4", [S, 256], dt.float32, kind="ExternalInput").ap()
    mask_d = nc.dram_tensor("mask", [512, 512], dt.bfloat16, kind="ExternalInput").ap()
    out_d = nc.dram_tensor("out", [S, DIM], dt.float32, kind="ExternalOutput").ap()

    with tile.TileContext(nc) as tc:
        with tc.For_i(0, reps, 1) if reps else nullcontext(), tc.tile_pool(
            name="const", bufs=1
        ) as cp:
            wt_sb = cp.tile([128, CB, 768], dt.bfloat16)
            nc.sync.dma_start(out=wt_sb, in_=wt_d.rearrange("(cb c) n -> c cb n", c=128))
            cs4_sb = cp.tile([128, SB, 256], dt.float32)
            nc.sync.dma_start(out=cs4_sb, in_=cs4_d.rearrange("(sb p) n -> p sb n", p=128))
            sn4_sb = cp.tile([128, SB, 256], dt.float32)
            nc.sync.dma_start(out=sn4_sb, in_=sn4_d.rearrange("(sb p) n -> p sb n", p=128))
            mask_sb = cp.tile([128, 4, 512], dt.bfloat16)
            nc.sync.dma_start(out=mask_sb, in_=mask_d.rearrange("(m p) n -> p m n", p=128))
            ones_sb = cp.tile([128, 128], dt.bfloat16)
            nc.vector.memset(ones_sb, 1.0)
            ptc_sb = cp.tile([128, 512], dt.bfloat16)
            nc.vector.memset(ptc_sb, 0.5)
            ident_sb = cp.tile([128, 128], dt.bfloat16)
            make_identity(nc, ident_sb)

            qt_sb = cp.tile([128, QH_PER_CORE, S], dt.bfloat16)  # [d, h, s]
            kt_sb = cp.tile([128, S], dt.bfloat16)  # [d, s]
            v_sb = cp.tile([128, SB, HD], dt.bfloat16)  # [s128, sb, d]

            # ---------------- phase A: projections + rope + transposes --------
            with (
                tc.tile_pool(name="pa", bufs=2) as pa,
                tc.tile_pool(name="pap", bufs=2, space="PSUM") as pap,
            ):
                for sb in range(SB):
                    if sb % 4 == 0:
                        xt_sb = pa.tile([128, CB, 512], dt.bfloat16, tag="xt")
                        nc.sync.dma_start(
                            out=xt_sb,
                            in_=xt_d.rearrange("(cb c) s -> c cb s", c=128)[
                                :, :, (sb // 4) * 512 : (sb // 4 + 1) * 512
                            ],
                        )
                    sbl = sb % 4
                    ps = pap.tile([128, 768], dt.float32, tag="proj")
                    for cb in range(CB if "proj" in stages else 0):
                        lhsT = xt_sb[:, cb, sbl * 128 : (sbl + 1) * 128]
                        nc.tensor.matmul(
                            ps[:, 0:512],
                            lhsT=lhsT,
                            rhs=wt_sb[:, cb, 0:512],
                            start=(cb == 0),
                            stop=(cb == CB - 1),
                        )
                        nc.tensor.matmul(
                            ps[:, 512:768],
                            lhsT=lhsT,
                            rhs=wt_sb[:, cb, 512:768],
                            start=(cb == 0),
                            stop=(cb == CB - 1),
                        )
                    # rope Q (4 heads at once) then K, in [s, d] layout
                    do_rope = "rope" in stages
                    q_sb = pa.tile([128, DQ], dt.bfloat16, tag="q")
                    k_sb = pa.tile([128, HD], dt.bfloat16, tag="k")
                    t1 = pa.tile([128, 256], dt.float32, tag="t1")
                    t2 = pa.tile([128, 256], dt.float32, tag="t2")
                    c4 = cs4_sb[:, sb, :]
                    s4 = sn4_sb[:, sb, :]
                    a = ps[:, 0:512:2]
                    b = ps[:, 1:512:2]
                    do_rope and nc.vector.tensor_mul(t1, a, c4)
                    do_rope and nc.vector.tensor_mul(t2, b, s4)
                    do_rope and nc.vector.tensor_sub(q_sb[:, 0:512:2], t1, t2)
                    do_rope and nc.vector.tensor_mul(t1, a, s4)
                    do_rope and nc.vector.tensor_mul(t2, b, c4)
                    do_rope and nc.vector.tensor_add(q_sb[:, 1:512:2], t1, t2)
                    ak = ps[:, 512:640:2]
                    bk = ps[:, 513:640:2]
                    t1k = t1[:, 0:64]
                    t2k = t2[:, 0:64]
                    c4k = c4[:, 0:64]
                    s4k = s4[:, 0:64]
                    do_rope and nc.vector.tensor_mul(t1k, ak, c4k)
                    do_rope and nc.vector.tensor_mul(t2k, bk, s4k)
                    do_rope and nc.vector.tensor_sub(k_sb[:, 0:128:2], t1k, t2k)
                    do_rope and nc.vector.tensor_mul(t1k, ak, s4k)
                    do_rope and nc.vector.tensor_mul(t2k, bk, c4k)
                    do_rope and nc.vector.tensor_add(k_sb[:, 1:128:2], t1k, t2k)
                    # V: straight cast copy
                    do_rope and nc.vector.tensor_copy(v_sb[:, sb, :], ps[:, 640:768])
                    # transposes into resident QT / KT
                    for h in range(QH_PER_CORE if "tpose" in stages else 0):
                        pst = pap.tile([128, 128], dt.bfloat16, tag="tp", bufs=4)
                        nc.tensor.transpose(
                            pst, q_sb[:, h * 128 : (h + 1) * 128], ident_sb
                        )
                        nc.vector.tensor_copy(
                            qt_sb[:, h, sb * 128 : (sb + 1) * 128], pst
                        )
                    if "tpose" in stages:
                        pst = pap.tile([128, 128], dt.bfloat16, tag="tp", bufs=4)
                        nc.tensor.transpose(pst, k_sb, ident_sb)
                        nc.vector.tensor_copy(kt_sb[:, sb * 128 : (sb + 1) * 128], pst)

            # ---------------- phase B: attention + output projection ----------
            # Software-pipelined: the o-projection for chunk t-1 is emitted
            # between the per-head attention groups of chunk t, so PE has
            # dense matmul work while ACT runs the exps of the current chunk.
            with (
                tc.tile_pool(name="pb", bufs=2) as pb,
                tc.tile_pool(name="pbp", bufs=2, space="PSUM") as pbp,
            ):
                woT_sb = pb.tile([128, QH_PER_CORE, DIM], dt.bfloat16, bufs=1)
                nc.sync.dma_start(
                    out=woT_sb, in_=wot_d.rearrange("(db p) e -> p db e", p=128)
                )

                def oproj_group(t, sbl, ats):
                    if "oproj" not in stages:
                        return
                    o_sb = pb.tile([128, DIM], dt.float32, tag="osb")
                    for e in range(ET):
                        ps_out = pbp.tile([128, 512], dt.float32, tag="oproj")
                        for h in range(QH_PER_CORE):
                            nc.tensor.matmul(
                                ps_out,
                                lhsT=ats[h][:, sbl * 128 : (sbl + 1) * 128],
                                rhs=woT_sb[:, h, e * 512 : (e + 1) * 512],
                                start=(h == 0),
                                stop=(h == QH_PER_CORE - 1),
                            )
                        nc.vector.tensor_copy(o_sb[:, e * 512 : (e + 1) * 512], ps_out)
                    if not NO_OUTDMA:
                        nc.scalar.dma_start(
                            out=out_d[(4 * t + sbl) * 128 : (4 * t + sbl + 1) * 128, :],
                            in_=o_sb,
                        )

                prev_ats = None
                for t in range(NT if ("scores" in stages) else 0):
                    nkb = 4 * (t + 1)
                    at_tiles = []
                    for h in range(QH_PER_CORE):
                        qs = qt_sb[:, h, t * 512 : (t + 1) * 512]
                        ps_o = pbp.tile([128, 512], dt.float32, tag="attnT", bufs=1)
                        ps_d = pbp.tile([128, 512], dt.float32, tag="denom", bufs=1)
                        for kb in range(0, nkb, 2):
                            ps_s = pbp.tile([128, 1024], dt.float32, tag="scores")
                            for j in (0, 1):
                                nc.tensor.matmul(
                                    ps_s[:, j * 512 : (j + 1) * 512],
                                    lhsT=kt_sb[:, (kb + j) * 128 : (kb + j + 1) * 128],
                                    rhs=qs,
                                    start=True,
                                    stop=True,
                                )
                            if PT_CONST:
                                pt = ptc_sb
                            else:
                                pt = pb.tile(
                                    [128, 1024], dt.bfloat16, tag="pt", bufs=4
                                )
                                if not NO_EXP:
                                    nc.scalar.activation(pt, ps_s, Exp, scale=SCALE)
                                else:
                                    nc.gpsimd.memset(pt, 0.5)
                                for j in (0, 1):
                                    if kb + j >= 4 * t:
                                        nc.vector.tensor_mul(
                                            pt[:, j * 512 : (j + 1) * 512],
                                            pt[:, j * 512 : (j + 1) * 512],
                                            mask_sb[:, kb + j - 4 * t, :],
                                        )
                            if "pv" in stages:
                                for j in (0, 1):
                                    ptj = pt[:, j * 512 : (j + 1) * 512] if not PT_CONST else ptc_sb
                                    nc.tensor.matmul(
                                        ps_o,
                                        lhsT=v_sb[:, kb + j, :],
                                        rhs=ptj,
                                        start=(kb + j == 0),
                                        stop=(kb + j == nkb - 1),
                                    )
                                    nc.tensor.matmul(
                                        ps_d,
                                        lhsT=ones_sb,
                                        rhs=ptj,
                                        start=(kb + j == 0),
                                        stop=(kb + j == nkb - 1),
                                    )
                        at = pb.tile([128, 512], dt.bfloat16, tag=f"at{h}")
                        if "pv" in stages:
                            recip = pb.tile([128, 512], dt.float32, tag="recip")
                            nc.vector.reciprocal(recip, ps_d)
                            nc.vector.tensor_mul(at, ps_o, recip)
                        at_tiles.append(at)
                        if prev_ats is not None:
                            oproj_group(t - 1, h, prev_ats)
                    prev_ats = at_tiles
                if prev_ats is not None:
                    for sbl in range(4):
                        oproj_group(NT - 1, sbl, prev_ats)
    nc.compile()
    return nc


def _prep_inputs(x, cos, sin, wq, wk, wv, wo):
    x = np.asarray(x, dtype=np.float32)
    cos = np.asarray(cos, dtype=np.float32)
    sin = np.asarray(sin, dtype=np.float32)
    wq = np.asarray(wq, dtype=np.float32)
    wk = np.asarray(wk, dtype=np.float32)
    wv = np.asarray(wv, dtype=np.float32)
    wo = np.asarray(wo, dtype=np.float32)

    xt = np.ascontiguousarray(x[0].T).astype(bf16)  # [DIM, S]
    cs4 = np.ascontiguousarray(np.tile(cos, (1, 4)))  # [S, 256] f32
    sn4 = np.ascontiguousarray(np.tile(sin, (1, 4)))

    # causal masks for the 4 diagonal sub-blocks: mask[r, c] = (r + delta) <= c
    r = np.arange(128)[:, None]
    c = np.arange(512)[None, :]
    mask = np.concatenate(
        [((r + d) <= c).astype(bf16) for d in (0, 128, 256, 384)], axis=0
    )  # [512, 512]

    in_maps = []
    for i in range(N_CORES):
        wq_i = wq[DQ * i : DQ * (i + 1)]  # [512, DIM]
        wk_i = wk[HD * i : HD * (i + 1)]  # [128, DIM]
        wv_i = wv[HD * i : HD * (i + 1)]
        wt = np.concatenate([wq_i.T, wk_i.T, wv_i.T], axis=1).astype(bf16)  # [DIM,768]
        wot = np.ascontiguousarray(wo[:, DQ * i : DQ * (i + 1)].T).astype(
            bf16
        )  # [512, DIM]
        in_maps.append(
            {
                "xt": xt,
                "wt": np.ascontiguousarray(wt),
                "wot": wot,
                "cs4": cs4,
                "sn4": sn4,
                "mask": np.ascontiguousarray(mask),
            }
        )
    return in_maps


def _get_runner():
    global _RUNNER
    if _RUNNER is None:
        _RUNNER = _build()
    return _RUNNER


def kernel(x, cos, sin, wq, wk, wv, wo):
    from concourse.bass_utils import run_bass_kernel_spmd

    nc = _get_runner()
    in_maps = _prep_inputs(x, cos, sin, wq, wk, wv, wo)
    res = run_bass_kernel_spmd(nc, in_maps, list(range(N_CORES)))
    out = np.zeros((S, DIM), dtype=np.float32)
    for i in range(N_CORES):
        out += res.results[i]["out"]
    return out[None].astype(np.float32)


# revision 10
# speedup vs baseline: 137.5061x; 1.0578x over previous
"""Trainium2 Bass kernel for GQA attention prefill (Mistral-style, RoPE, causal).

B=1, S=2048, DIM=4096, 32 Q heads / 8 KV heads, HD=128, rope theta 1e6.

Sharding: tensor-parallel over heads across 8 cores. Core i gets Q heads
4i..4i+3 and KV head i. x is replicated (pre-transposed + bf16-cast on host).
Each core computes its 4 heads' attention and a partial output projection
(contraction over its 512 input dims of wo); the host sums the 8 partials.

Per-core dataflow (all matmuls bf16 with fp32 PSUM accumulation):
  phase A (per 128-row s block):
    xT tiles [c,s] (lhsT) x wT [c, q|k|v] (rhs) -> psum [s, 768]
    rope applied in [s, d] layout via stride-2 APs (DVE), cast bf16
    PE-transpose q/k 128x128 blocks -> resident QT/KT [d, s]; V kept [s, d]
  phase B (per 512-col q chunk t, per head h):
    scores_T [k,q] = KT_tile.T @ QT  (one matmul per 128-k tile, no accum)
    P_T = exp(scale * scores_T) on ACT (no max subtraction: |scores| < ~15),
    diagonal blocks masked by precomputed 0/1 tiles (DVE)
    attn_T [d, q] accumulated via lhsT=V tiles; denom via lhsT=ones
    (every row of the denom psum equals the column sum of P_T)
    normalize on DVE (reciprocal + multiply) -> at [d', s] bf16
  o-proj: psum [s,512e] accumulated over the 4 heads, lhsT=at slices,
    rhs=woT [d', e]; evacuate fp32 and DMA to the partial output.
"""

import numpy as np
import ml_dtypes

S = 2048
DIM = 4096
HD = 128
N_CORES = 8
QH_PER_CORE = 4  # 512 q dims per core
DQ = QH_PER_CORE * HD  # 512
SCALE = 1.0 / float(np.sqrt(HD))
SB = S // 128  # 16 s blocks
CB = DIM // 128  # 32 contraction blocks
NT = S // 512  # 4 q chunks
ET = DIM // 512  # 8 e tiles

bf16 = ml_dtypes.bfloat16

_RUNNER = None


ALL_STAGES = frozenset({"proj", "rope", "tpose", "scores", "pv", "oproj"})

# debug knobs for perf isolation (set by bench scripts)
NO_EXP = False
NO_OUTDMA = False
PT_CONST = False


def _build(reps=None, stages=ALL_STAGES):
    import concourse.bass as bass
    import concourse.mybir as mybir
    import concourse.tile as tile
    from concourse import bacc
    from concourse.masks import make_identity
    from contextlib import nullcontext

    dt = mybir.dt
    Exp = mybir.ActivationFunctionType.Exp

    nc = bacc.Bacc(
        "TRN2", target_bir_lowering=False, debug=False, num_devices=N_CORES
    )

    xt_d = nc.dram_tensor("xt", [DIM, S], dt.bfloat16, kind="ExternalInput").ap()
    wt_d = nc.dram_tensor("wt", [DIM, 768], dt.bfloat16, kind="ExternalInput").ap()
    wot_d = nc.dram_tensor("wot", [DQ, DIM], dt.bfloat16, kind="ExternalInput").ap()
    cs4_d = nc.dram_tensor("cs4", [S, 256], dt.float32, kind="ExternalInput").ap()
    sn4_d = nc.dram_tensor("sn4", [S, 256], dt.float32, kind="ExternalInput").ap()
    mask_d = nc.dram_tensor("mask", [512, 512], dt.bfloat16, kind="ExternalInput").ap()
    out_d = nc.dram_tensor("out", [S, DIM], dt.float32, kind="ExternalOutput").ap()

    with tile.TileContext(nc) as tc:
        with tc.For_i(0, reps, 1) if reps else nullcontext(), tc.tile_pool(
            name="const", bufs=1
        ) as cp:
            wt_sb = cp.tile([128, CB, 768], dt.bfloat16)
            nc.sync.dma_start(out=wt_sb, in_=wt_d.rearrange("(cb c) n -> c cb n", c=128))
            cs4_sb = cp.tile([128, SB, 256], dt.float32)
            nc.sync.dma_start(out=cs4_sb, in_=cs4_d.rearrange("(sb p) n -> p sb n", p=128))
            sn4_sb = cp.tile([128, SB, 256], dt.float32)
            nc.sync.dma_start(out=sn4_sb, in_=sn4_d.rearrange("(sb p) n -> p sb n", p=128))
            mask_sb = cp.tile([128, 4, 512], dt.bfloat16)
            nc.sync.dma_start(out=mask_sb, in_=mask_d.rearrange("(m p) n -> p m n", p=128))
            ones_sb = cp.tile([128, 128], dt.float32)
            nc.vector.memset(ones_sb, 1.0)
            ptc_sb = cp.tile([128, 512], dt.bfloat16)
            nc.vector.memset(ptc_sb, 0.5)
            ident_sb = cp.tile([128, 128], dt.bfloat16)
            make_identity(nc, ident_sb)

            qt_sb = cp.tile([128, QH_PER_CORE, S], dt.bfloat16)  # [d, h, s]
            kt_sb = cp.tile([128, S], dt.bfloat16)  # [d, s]
            v_sb = cp.tile([128, SB, HD], dt.bfloat16)  # [s128, sb, d]

            # ---------------- phase A: projections + rope + transposes --------
            with (
                tc.tile_pool(name="pa", bufs=2) as pa,
                tc.tile_pool(name="pap", bufs=2, space="PSUM") as pap,
            ):
                for sb in range(SB):
                    if sb % 4 == 0:
                        xt_sb = pa.tile([128, CB, 512], dt.bfloat16, tag="xt")
                        nc.sync.dma_start(
                            out=xt_sb,
                            in_=xt_d.rearrange("(cb c) s -> c cb s", c=128)[
                                :, :, (sb // 4) * 512 : (sb // 4 + 1) * 512
                            ],
                        )
                    sbl = sb % 4
                    ps = pap.tile([128, 768], dt.float32, tag="proj")
                    for cb in range(CB if "proj" in stages else 0):
                        lhsT = xt_sb[:, cb, sbl * 128 : (sbl + 1) * 128]
                        nc.tensor.matmul(
                            ps[:, 0:512],
                            lhsT=lhsT,
                            rhs=wt_sb[:, cb, 0:512],
                            start=(cb == 0),
                            stop=(cb == CB - 1),
                        )
                        nc.tensor.matmul(
                            ps[:, 512:768],
                            lhsT=lhsT,
                            rhs=wt_sb[:, cb, 512:768],
                            start=(cb == 0),
                            stop=(cb == CB - 1),
                        )
                    # rope Q (4 heads at once) then K, in [s, d] layout
                    do_rope = "rope" in stages
                    q_sb = pa.tile([128, DQ], dt.bfloat16, tag="q")
                    k_sb = pa.tile([128, HD], dt.bfloat16, tag="k")
                    t1 = pa.tile([128, 256], dt.float32, tag="t1")
                    t2 = pa.tile([128, 256], dt.float32, tag="t2")
                    c4 = cs4_sb[:, sb, :]
                    s4 = sn4_sb[:, sb, :]
                    a = ps[:, 0:512:2]
                    b = ps[:, 1:512:2]
                    do_rope and nc.vector.tensor_mul(t1, a, c4)
                    do_rope and nc.vector.tensor_mul(t2, b, s4)
                    do_rope and nc.vector.tensor_sub(q_sb[:, 0:512:2], t1, t2)
                    do_rope and nc.vector.tensor_mul(t1, a, s4)
                    do_rope and nc.vector.tensor_mul(t2, b, c4)
                    do_rope and nc.vector.tensor_add(q_sb[:, 1:512:2], t1, t2)
                    ak = ps[:, 512:640:2]
                    bk = ps[:, 513:640:2]
                    t1k = t1[:, 0:64]
                    t2k = t2[:, 0:64]
                    c4k = c4[:, 0:64]
                    s4k = s4[:, 0:64]
                    do_rope and nc.vector.tensor_mul(t1k, ak, c4k)
                    do_rope and nc.vector.tensor_mul(t2k, bk, s4k)
                    do_rope and nc.vector.tensor_sub(k_sb[:, 0:128:2], t1k, t2k)
                    do_rope and nc.vector.tensor_mul(t1k, ak, s4k)
                    do_rope and nc.vector.tensor_mul(t2k, bk, c4k)
                    do_rope and nc.vector.tensor_add(k_sb[:, 1:128:2], t1k, t2k)
                    # V: straight cast copy
                    do_rope and nc.vector.tensor_copy(v_sb[:, sb, :], ps[:, 640:768])
                    # transposes into resident QT / KT
                    for h in range(QH_PER_CORE if "tpose" in stages else 0):
                        pst = pap.tile([128, 128], dt.bfloat16, tag="tp", bufs=4)
                        nc.tensor.transpose(
                            pst, q_sb[:, h * 128 : (h + 1) * 128], ident_sb
                        )
                        nc.vector.tensor_copy(
                            qt_sb[:, h, sb * 128 : (sb + 1) * 128], pst
                        )
                    if "tpose" in stages:
                        pst = pap.tile([128, 128], dt.bfloat16, tag="tp", bufs=4)
                        nc.tensor.transpose(pst, k_sb, ident_sb)
                        nc.vector.tensor_copy(kt_sb[:, sb * 128 : (sb + 1) * 128], pst)

            # ---------------- phase B: attention + output projection ----------
            # Software-pipelined: the o-projection for chunk t-1 is emitted
            # between the per-head attention groups of chunk t, so PE has
            # dense matmul work while ACT runs the exps of the current chunk.
            with (
                tc.tile_pool(name="pb", bufs=2) as pb,
                tc.tile_pool(name="pbp", bufs=2, space="PSUM") as pbp,
            ):
                woT_sb = pb.tile([128, QH_PER_CORE, DIM], dt.bfloat16, bufs=1)
                nc.sync.dma_start(
                    out=woT_sb, in_=wot_d.rearrange("(db p) e -> p db e", p=128)
                )

                def oproj_group(t, sbl, ats):
                    if "oproj" not in stages:
                        return
                    o_sb = pb.tile([128, DIM], dt.float32, tag="osb")
                    for e in range(ET):
                        ps_out = pbp.tile([128, 512], dt.float32, tag="oproj")
                        for h in range(QH_PER_CORE):
                            nc.tensor.matmul(
                                ps_out,
                                lhsT=ats[h][:, sbl * 128 : (sbl + 1) * 128],
                                rhs=woT_sb[:, h, e * 512 : (e + 1) * 512],
                                start=(h == 0),
                                stop=(h == QH_PER_CORE - 1),
                            )
                        nc.vector.tensor_copy(o_sb[:, e * 512 : (e + 1) * 512], ps_out)
                    if not NO_OUTDMA:
                        nc.scalar.dma_start(
                            out=out_d[(4 * t + sbl) * 128 : (4 * t + sbl + 1) * 128, :],
                            in_=o_sb,
                        )

                prev_ats = None
                for t in range(NT if ("scores" in stages) else 0):
                    nkb = 4 * (t + 1)
                    at_tiles = []
                    for h in range(QH_PER_CORE):
                        qs = qt_sb[:, h, t * 512 : (t + 1) * 512]
                        ps_o = pbp.tile([128, 512], dt.float32, tag="attnT", bufs=1)
                        dacc = pb.tile([128, 512], dt.float32, tag="dacc", bufs=2)
                        for kb in range(0, nkb, 2):
                            ps_s = pbp.tile([128, 1024], dt.float32, tag="scores")
                            for j in (0, 1):
                                nc.tensor.matmul(
                                    ps_s[:, j * 512 : (j + 1) * 512],
                                    lhsT=kt_sb[:, (kb + j) * 128 : (kb + j + 1) * 128],
                                    rhs=qs,
                                    start=True,
                                    stop=True,
                                )
                            if PT_CONST:
                                pt = ptc_sb
                            else:
                                pt = pb.tile(
                                    [128, 1024], dt.bfloat16, tag="pt", bufs=4
                                )
                                if not NO_EXP:
                                    nc.scalar.activation(pt, ps_s, Exp, scale=SCALE)
                                else:
                                    nc.gpsimd.memset(pt, 0.5)
                                for j in (0, 1):
                                    if kb + j >= 4 * t:
                                        nc.vector.tensor_mul(
                                            pt[:, j * 512 : (j + 1) * 512],
                                            pt[:, j * 512 : (j + 1) * 512],
                                            mask_sb[:, kb + j - 4 * t, :],
                                        )
                            if "pv" in stages:
                                for j in (0, 1):
                                    ptj = pt[:, j * 512 : (j + 1) * 512] if not PT_CONST else ptc_sb
                                    nc.tensor.matmul(
                                        ps_o,
                                        lhsT=v_sb[:, kb + j, :],
                                        rhs=ptj,
                                        start=(kb + j == 0),
                                        stop=(kb + j == nkb - 1),
                                    )
                                    # denominator partials accumulate on DVE
                                    if kb + j == 0:
                                        nc.vector.tensor_copy(dacc, ptj)
                                    else:
                                        nc.vector.tensor_add(dacc, dacc, ptj)
                        at = pb.tile([128, 512], dt.bfloat16, tag=f"at{h}")
                        if "pv" in stages:
                            # partition-reduce + broadcast the denominator in
                            # one fp32 matmul: every output row = column sum
                            ps_d = pbp.tile([128, 512], dt.float32, tag="denom", bufs=1)
                            nc.tensor.matmul(
                                ps_d, lhsT=ones_sb, rhs=dacc, start=True, stop=True
                            )
                            recip = pb.tile([128, 512], dt.float32, tag="recip")
                            nc.vector.reciprocal(recip, ps_d)
                            nc.vector.tensor_mul(at, ps_o, recip)
                        at_tiles.append(at)
                        if prev_ats is not None:
                            oproj_group(t - 1, h, prev_ats)
                    prev_ats = at_tiles
                if prev_ats is not None:
                    for sbl in range(4):
                        oproj_group(NT - 1, sbl, prev_ats)
    nc.compile()
    return nc


def _prep_inputs(x, cos, sin, wq, wk, wv, wo):
    x = np.asarray(x, dtype=np.float32)
    cos = np.asarray(cos, dtype=np.float32)
    sin = np.asarray(sin, dtype=np.float32)
    wq = np.asarray(wq, dtype=np.float32)
    wk = np.asarray(wk, dtype=np.float32)
    wv = np.asarray(wv, dtype=np.float32)
    wo = np.asarray(wo, dtype=np.float32)

    xt = np.ascontiguousarray(x[0].T).astype(bf16)  # [DIM, S]
    cs4 = np.ascontiguousarray(np.tile(cos, (1, 4)))  # [S, 256] f32
    sn4 = np.ascontiguousarray(np.tile(sin, (1, 4)))

    # causal masks for the 4 diagonal sub-blocks: mask[r, c] = (r + delta) <= c
    r = np.arange(128)[:, None]
    c = np.arange(512)[None, :]
    mask = np.concatenate(
        [((r + d) <= c).astype(bf16) for d in (0, 128, 256, 384)], axis=0
    )  # [512, 512]

    in_maps = []
    for i in range(N_CORES):
        wq_i = wq[DQ * i : DQ * (i + 1)]  # [512, DIM]
        wk_i = wk[HD * i : HD * (i + 1)]  # [128, DIM]
        wv_i = wv[HD * i : HD * (i + 1)]
        wt = np.concatenate([wq_i.T, wk_i.T, wv_i.T], axis=1).astype(bf16)  # [DIM,768]
        wot = np.ascontiguousarray(wo[:, DQ * i : DQ * (i + 1)].T).astype(
            bf16
        )  # [512, DIM]
        in_maps.append(
            {
                "xt": xt,
                "wt": np.ascontiguousarray(wt),
                "wot": wot,
                "cs4": cs4,
                "sn4": sn4,
                "mask": np.ascontiguousarray(mask),
            }
        )
    return in_maps


def _get_runner():
    global _RUNNER
    if _RUNNER is None:
        _RUNNER = _build()
    return _RUNNER


def kernel(x, cos, sin, wq, wk, wv, wo):
    from concourse.bass_utils import run_bass_kernel_spmd

    nc = _get_runner()
    in_maps = _prep_inputs(x, cos, sin, wq, wk, wv, wo)
    res = run_bass_kernel_spmd(nc, in_maps, list(range(N_CORES)))
    out = np.zeros((S, DIM), dtype=np.float32)
    for i in range(N_CORES):
        out += res.results[i]["out"]
    return out[None].astype(np.float32)


# revision 11
# speedup vs baseline: 139.1982x; 1.0123x over previous
"""Trainium2 Bass kernel for GQA attention prefill (Mistral-style, RoPE, causal).

B=1, S=2048, DIM=4096, 32 Q heads / 8 KV heads, HD=128, rope theta 1e6.

Sharding: tensor-parallel over heads across 8 cores. Core i gets Q heads
4i..4i+3 and KV head i. x is replicated (pre-transposed + bf16-cast on host).
Each core computes its 4 heads' attention and a partial output projection
(contraction over its 512 input dims of wo); the host sums the 8 partials.

Per-core dataflow (all matmuls bf16 with fp32 PSUM accumulation):
  phase A (per 128-row s block):
    xT tiles [c,s] (lhsT) x wT [c, q|k|v] (rhs) -> psum [s, 768]
    rope applied in [s, d] layout via stride-2 APs (DVE), cast bf16
    PE-transpose q/k 128x128 blocks -> resident QT/KT [d, s]; V kept [s, d]
  phase B (per 512-col q chunk t, per head h):
    scores_T [k,q] = KT_tile.T @ QT  (one matmul per 128-k tile, no accum)
    P_T = exp(scale * scores_T) on ACT (no max subtraction: |scores| < ~15),
    diagonal blocks masked by precomputed 0/1 tiles (DVE)
    attn_T [d, q] accumulated via lhsT=V tiles; denom via lhsT=ones
    (every row of the denom psum equals the column sum of P_T)
    normalize on DVE (reciprocal + multiply) -> at [d', s] bf16
  o-proj: psum [s,512e] accumulated over the 4 heads, lhsT=at slices,
    rhs=woT [d', e]; evacuate fp32 and DMA to the partial output.
"""

import numpy as np
import ml_dtypes

S = 2048
DIM = 4096
HD = 128
N_CORES = 8
QH_PER_CORE = 4  # 512 q dims per core
DQ = QH_PER_CORE * HD  # 512
SCALE = 1.0 / float(np.sqrt(HD))
SB = S // 128  # 16 s blocks
CB = DIM // 128  # 32 contraction blocks
NT = S // 512  # 4 q chunks
ET = DIM // 512  # 8 e tiles

bf16 = ml_dtypes.bfloat16

_RUNNER = None


ALL_STAGES = frozenset({"proj", "rope", "tpose", "scores", "pv", "oproj"})

# debug knobs for perf isolation (set by bench scripts)
NO_EXP = False
NO_OUTDMA = False
PT_CONST = False


def _build(reps=None, stages=ALL_STAGES):
    import concourse.bass as bass
    import concourse.mybir as mybir
    import concourse.tile as tile
    from concourse import bacc
    from concourse.masks import make_identity
    from contextlib import nullcontext

    dt = mybir.dt
    Exp = mybir.ActivationFunctionType.Exp

    nc = bacc.Bacc(
        "TRN2", target_bir_lowering=False, debug=False, num_devices=N_CORES
    )

    xt_d = nc.dram_tensor("xt", [DIM, S], dt.bfloat16, kind="ExternalInput").ap()
    wt_d = nc.dram_tensor("wt", [DIM, 768], dt.bfloat16, kind="ExternalInput").ap()
    wot_d = nc.dram_tensor("wot", [DQ, DIM], dt.bfloat16, kind="ExternalInput").ap()
    csd_d = nc.dram_tensor("csd", [128, S], dt.float32, kind="ExternalInput").ap()
    snd_d = nc.dram_tensor("snd", [128, S], dt.float32, kind="ExternalInput").ap()
    mask_d = nc.dram_tensor("mask", [512, 512], dt.bfloat16, kind="ExternalInput").ap()
    out_d = nc.dram_tensor("out", [S, DIM], dt.float32, kind="ExternalOutput").ap()

    with tile.TileContext(nc) as tc:
        with tc.For_i(0, reps, 1) if reps else nullcontext(), tc.tile_pool(
            name="const", bufs=1
        ) as cp:
            mask_sb = cp.tile([128, 4, 512], dt.bfloat16)
            nc.sync.dma_start(out=mask_sb, in_=mask_d.rearrange("(m p) n -> p m n", p=128))
            ones_sb = cp.tile([128, 128], dt.float32)
            nc.vector.memset(ones_sb, 1.0)
            ptc_sb = cp.tile([128, 512], dt.bfloat16)
            nc.vector.memset(ptc_sb, 0.5)
            ident_sb = cp.tile([128, 128], dt.bfloat16)
            make_identity(nc, ident_sb)

            qt_sb = cp.tile([128, QH_PER_CORE, S], dt.bfloat16)  # [d, h, s]
            kt_sb = cp.tile([128, S], dt.bfloat16)  # [d, s]
            v_sb = cp.tile([128, SB, HD], dt.bfloat16)  # [s128, sb, d]

            # ---------------- phase A: projections + rope (direct QT) ---------
            # Weights are the stationary operand; psum comes out as [d, s]
            # (already transposed for attention). Q/K rows are host-permuted
            # per head into [even-pairs | odd-pairs] so rope works on
            # partition halves (inputs share a base; outputs may shift).
            # V is PE-transposed back to [s, d] (16 blocks).
            with (
                tc.tile_pool(name="pa", bufs=2) as pa,
                tc.tile_pool(name="pap", bufs=3, space="PSUM") as pap,
            ):
                wt_sb = pa.tile([128, CB, 768], dt.bfloat16, bufs=1)
                nc.sync.dma_start(
                    out=wt_sb, in_=wt_d.rearrange("(cb c) n -> c cb n", c=128)
                )
                csd_sb = pa.tile([128, S], dt.float32, bufs=1)
                nc.sync.dma_start(out=csd_sb, in_=csd_d)
                snd_sb = pa.tile([128, S], dt.float32, bufs=1)
                nc.sync.dma_start(out=snd_sb, in_=snd_d)

                def rope_evac(ps, dest, s0):
                    # dest[0:64]   = a*cos - b*sin
                    # dest[64:128] = a*sin + b*cos   (a=rows 0:64, b=rows 64:128)
                    cs = csd_sb[:, s0 : s0 + 512]
                    sn = snd_sb[:, s0 : s0 + 512]
                    t1 = pa.tile([128, 512], dt.float32, tag="t1")
                    t2 = pa.tile([128, 512], dt.float32, tag="t2")
                    nc.vector.tensor_mul(t1, ps, cs)
                    nc.vector.tensor_mul(t2[0:64, :], ps[64:128, :], sn[64:128, :])
                    nc.vector.tensor_mul(t2[64:128, :], ps[0:64, :], sn[0:64, :])
                    nc.vector.tensor_sub(dest[0:64, :], t1[0:64, :], t2[0:64, :])
                    nc.vector.tensor_add(dest[64:128, :], t1[64:128, :], t2[64:128, :])

                for sc in range(4):  # s chunks of 512
                    s0 = sc * 512
                    xt_sb = pa.tile([128, CB, 512], dt.bfloat16, tag="xt")
                    nc.sync.dma_start(
                        out=xt_sb,
                        in_=xt_d.rearrange("(cb c) s -> c cb s", c=128)[
                            :, :, s0 : s0 + 512
                        ],
                    )
                    for dtile in range(6):  # 4 Q heads, K, V
                        ps = pap.tile([128, 512], dt.float32, tag="proj")
                        for cb in range(CB if "proj" in stages else 0):
                            nc.tensor.matmul(
                                ps,
                                lhsT=wt_sb[:, cb, dtile * 128 : (dtile + 1) * 128],
                                rhs=xt_sb[:, cb, :],
                                start=(cb == 0),
                                stop=(cb == CB - 1),
                            )
                        if "rope" not in stages:
                            continue
                        if dtile < QH_PER_CORE:
                            rope_evac(ps, qt_sb[:, dtile, s0 : s0 + 512], s0)
                        elif dtile == QH_PER_CORE:
                            rope_evac(ps, kt_sb[:, s0 : s0 + 512], s0)
                        else:
                            vt_st = pa.tile([128, 512], dt.bfloat16, tag="vt")
                            nc.vector.tensor_copy(vt_st, ps)
                            for b in range(4 if "tpose" in stages else 0):
                                pst = pap.tile(
                                    [128, 128], dt.bfloat16, tag="tp", bufs=2
                                )
                                nc.tensor.transpose(
                                    pst, vt_st[:, b * 128 : (b + 1) * 128], ident_sb
                                )
                                nc.vector.tensor_copy(
                                    v_sb[:, sc * 4 + b, :], pst
                                )

            # ---------------- phase B: attention + output projection ----------
            # Software-pipelined: the o-projection for chunk t-1 is emitted
            # between the per-head attention groups of chunk t, so PE has
            # dense matmul work while ACT runs the exps of the current chunk.
            with (
                tc.tile_pool(name="pb", bufs=2) as pb,
                tc.tile_pool(name="pbp", bufs=2, space="PSUM") as pbp,
            ):
                woT_sb = pb.tile([128, QH_PER_CORE, DIM], dt.bfloat16, bufs=1)
                nc.sync.dma_start(
                    out=woT_sb, in_=wot_d.rearrange("(db p) e -> p db e", p=128)
                )

                def oproj_group(t, sbl, ats):
                    if "oproj" not in stages:
                        return
                    o_sb = pb.tile([128, DIM], dt.float32, tag="osb")
                    for e in range(ET):
                        ps_out = pbp.tile([128, 512], dt.float32, tag="oproj")
                        for h in range(QH_PER_CORE):
                            nc.tensor.matmul(
                                ps_out,
                                lhsT=ats[h][:, sbl * 128 : (sbl + 1) * 128],
                                rhs=woT_sb[:, h, e * 512 : (e + 1) * 512],
                                start=(h == 0),
                                stop=(h == QH_PER_CORE - 1),
                            )
                        nc.vector.tensor_copy(o_sb[:, e * 512 : (e + 1) * 512], ps_out)
                    if not NO_OUTDMA:
                        nc.scalar.dma_start(
                            out=out_d[(4 * t + sbl) * 128 : (4 * t + sbl + 1) * 128, :],
                            in_=o_sb,
                        )

                prev_ats = None
                for t in range(NT if ("scores" in stages) else 0):
                    nkb = 4 * (t + 1)
                    at_tiles = []
                    for h in range(QH_PER_CORE):
                        qs = qt_sb[:, h, t * 512 : (t + 1) * 512]
                        ps_o = pbp.tile([128, 512], dt.float32, tag="attnT", bufs=1)
                        dacc = pb.tile([128, 512], dt.float32, tag="dacc", bufs=2)
                        for kb in range(0, nkb, 2):
                            ps_s = pbp.tile([128, 1024], dt.float32, tag="scores")
                            for j in (0, 1):
                                nc.tensor.matmul(
                                    ps_s[:, j * 512 : (j + 1) * 512],
                                    lhsT=kt_sb[:, (kb + j) * 128 : (kb + j + 1) * 128],
                                    rhs=qs,
                                    start=True,
                                    stop=True,
                                )
                            if PT_CONST:
                                pt = ptc_sb
                            else:
                                pt = pb.tile(
                                    [128, 1024], dt.bfloat16, tag="pt", bufs=4
                                )
                                if not NO_EXP:
                                    nc.scalar.activation(pt, ps_s, Exp, scale=SCALE)
                                else:
                                    nc.gpsimd.memset(pt, 0.5)
                                for j in (0, 1):
                                    if kb + j >= 4 * t:
                                        nc.vector.tensor_mul(
                                            pt[:, j * 512 : (j + 1) * 512],
                                            pt[:, j * 512 : (j + 1) * 512],
                                            mask_sb[:, kb + j - 4 * t, :],
                                        )
                            if "pv" in stages:
                                for j in (0, 1):
                                    ptj = pt[:, j * 512 : (j + 1) * 512] if not PT_CONST else ptc_sb
                                    nc.tensor.matmul(
                                        ps_o,
                                        lhsT=v_sb[:, kb + j, :],
                                        rhs=ptj,
                                        start=(kb + j == 0),
                                        stop=(kb + j == nkb - 1),
                                    )
                                    # denominator partials accumulate on DVE
                                    if kb + j == 0:
                                        nc.vector.tensor_copy(dacc, ptj)
                                    else:
                                        nc.vector.tensor_add(dacc, dacc, ptj)
                        at = pb.tile([128, 512], dt.bfloat16, tag=f"at{h}")
                        if "pv" in stages:
                            # partition-reduce + broadcast the denominator in
                            # one fp32 matmul: every output row = column sum
                            ps_d = pbp.tile([128, 512], dt.float32, tag="denom", bufs=1)
                            nc.tensor.matmul(
                                ps_d, lhsT=ones_sb, rhs=dacc, start=True, stop=True
                            )
                            recip = pb.tile([128, 512], dt.float32, tag="recip")
                            nc.vector.reciprocal(recip, ps_d)
                            nc.vector.tensor_mul(at, ps_o, recip)
                        at_tiles.append(at)
                        if prev_ats is not None:
                            oproj_group(t - 1, h, prev_ats)
                    prev_ats = at_tiles
                if prev_ats is not None:
                    for sbl in range(4):
                        oproj_group(NT - 1, sbl, prev_ats)
    nc.compile()
    return nc


def _prep_inputs(x, cos, sin, wq, wk, wv, wo):
    x = np.asarray(x, dtype=np.float32)
    cos = np.asarray(cos, dtype=np.float32)
    sin = np.asarray(sin, dtype=np.float32)
    wq = np.asarray(wq, dtype=np.float32)
    wk = np.asarray(wk, dtype=np.float32)
    wv = np.asarray(wv, dtype=np.float32)
    wo = np.asarray(wo, dtype=np.float32)

    xt = np.ascontiguousarray(x[0].T).astype(bf16)  # [DIM, S]
    # cos/sin transposed and duplicated into both partition halves [128, S]
    csd = np.ascontiguousarray(np.tile(cos.T, (2, 1)).astype(np.float32))
    snd = np.ascontiguousarray(np.tile(sin.T, (2, 1)).astype(np.float32))
    # de-interleave perm: head dim pairs (2i, 2i+1) -> rows (i, 64+i)
    perm = np.concatenate([np.arange(0, HD, 2), np.arange(1, HD, 2)])

    # causal masks for the 4 diagonal sub-blocks: mask[r, c] = (r + delta) <= c
    r = np.arange(128)[:, None]
    c = np.arange(512)[None, :]
    mask = np.concatenate(
        [((r + d) <= c).astype(bf16) for d in (0, 128, 256, 384)], axis=0
    )  # [512, 512]

    in_maps = []
    for i in range(N_CORES):
        wq_i = wq[DQ * i : DQ * (i + 1)]  # [512, DIM]
        wk_i = wk[HD * i : HD * (i + 1)]  # [128, DIM]
        wv_i = wv[HD * i : HD * (i + 1)]
        wq_p = wq_i.reshape(QH_PER_CORE, HD, DIM)[:, perm, :].reshape(DQ, DIM)
        wk_p = wk_i[perm, :]
        wt = np.concatenate([wq_p.T, wk_p.T, wv_i.T], axis=1).astype(bf16)
        wot = np.ascontiguousarray(wo[:, DQ * i : DQ * (i + 1)].T).astype(
            bf16
        )  # [512, DIM]
        in_maps.append(
            {
                "xt": xt,
                "wt": np.ascontiguousarray(wt),
                "wot": wot,
                "csd": csd,
                "snd": snd,
                "mask": np.ascontiguousarray(mask),
            }
        )
    return in_maps


def _get_runner():
    global _RUNNER
    if _RUNNER is None:
        _RUNNER = _build()
    return _RUNNER


def kernel(x, cos, sin, wq, wk, wv, wo):
    from concourse.bass_utils import run_bass_kernel_spmd

    nc = _get_runner()
    in_maps = _prep_inputs(x, cos, sin, wq, wk, wv, wo)
    res = run_bass_kernel_spmd(nc, in_maps, list(range(N_CORES)))
    out = np.zeros((S, DIM), dtype=np.float32)
    for i in range(N_CORES):
        out += res.results[i]["out"]
    return out[None].astype(np.float32)
